# revision 21
# baseline (speedup 1.0000x reference)
"""Trainium2 kernel for nn_DirectForce (gnn_message_passing).

Math (see reference):
    h   = softplus(X @ w1 + b1) - log(2)          per-edge MLP        [E, 64]
    mag = h @ w2 + b2                                                  [E, 1]
    mag = mag - mean_over_center(mag)[center]      scatter-mean debias
    pair-average mag between each directed edge and its reverse edge
    F   = segment_sum(mag * unit_vec, center)                          [N, 3]

The sorted-pair averaging pairs each directed edge with its reverse (same
length, negated vector), so the pair-averaged scatter reduces algebraically to
    F = segsum(0.5*mag*unit, center) - segsum(0.5*mag*unit, neigh)
which removes the argsort entirely.

Device design (8 NeuronCores, SPMD, edges partitioned contiguously 200k/core):
  - features pre-transposed + converted to bf16 on host (halves HBM traffic
    vs f32), pre-tiled to [NTILES, 128, XT_TILE] so every input DMA is
    contiguous; tiny MLP weights replicated (per the sharding hint).
  - edges processed in superchunks (SC) of 1024 edges = stacked z [128,512]
    (rows 0-63 = chunk-A hidden, 64-127 = chunk-B hidden via zero-padded
    [w1;0]/[0;w1] stationary weights); z groups of 2 SC (2 PSUM banks,
    bufs=3).  mm1 matmuls stay 512-wide: the matmul output must fit one
    PSUM bank (walrus ISA check s3d3_mm_num_elements rejects 1024; probed).
  - DVE (otherwise idle) copies each z group PSUM->SBUF into a per-tile
    staging buffer zs, freeing PSUM fast and decoupling PE from ACT.  ACT
    runs softplus as TWO wide ops per input tile: Exp in-place on zs (bias
    b1), Ln(zs+1) -> h (bf16).  ACT is the bottleneck; fewer/wider ACT ops
    cut its per-op overhead + semaphore traffic (~207us busy -> ~183us).
  - the activation-table patch pins Exp+Ln to the single table set holding
    both, so the ACT table loads exactly once.  (The "softplus_and_others"
    table set does NOT contain softplus on this stack - its anchor is the
    4-bucket act2 polynomial - so a 1-pass softplus is impossible; probed.)
  - mm2 burst for tile t-2 is emitted after tile t's mm1s (zs/h bufs=3), so
    PE's wait on LN never delays the copies ACT needs next.  Per SC one
    matmul [2,512] = w2-blockdiag^T @ h at partition offsets 0/32/64/96
    (tile_position) packs 4 SCs into ONE PSUM bank.
  - mag evacuation: DVE casts the bank [128,512] f32 -> bf16 in SBUF; DMA
    sends ONLY the 8 meaningful rows as TWO single-level strided transfers
    (rows 0:128:32 and 1:128:32) = 400KB/core instead of 6.4MB.  (A nested
    two-level partition AP mis-lowers: its 2nd level reads +512 elements
    into the next row - out of bounds; probed, corrupts the B half.)
Host post (index-structured tail, numpy): debias via bincount, unit
vectors, two segment sums.
"""

import numpy as np
import ml_dtypes

N_ATOMS = 50000
E_TOT = 1600000
D_FEAT = 128
H_DIM = 64
N_CORES = 8
EC = E_TOT // N_CORES          # 200000 edges per core
SC = 1024                      # edges per superchunk (2 chunks of 512)
GRP = 2                        # superchunks per z-tile (1024-wide mm1 moving)
NSC = 196                      # padded superchunk count (200000/1024 = 195.3)
ECP = NSC * SC                 # 200704 padded edges per core
NBANK = (NSC + 3) // 4         # 49 mag PSUM banks (4 SC each)

# input-tile taper in superchunks (one tile == one ACT Exp/Ln batch): ramp so
# the DMA stream stays ahead of ACT's consume rate; 10-SC tiles steady-state,
# small tail tile to shorten the end-of-kernel serial chain.
TILE_SIZES = [1, 2, 3, 3, 4, 5, 6, 6, 6, 6, 6] + [10] * 14 + [8]
assert sum(TILE_SIZES) == NSC
NTILES = len(TILE_SIZES)
XT_TILE = max(TILE_SIZES) * SC  # 10240

def _groups(sz):
    out = [GRP] * (sz // GRP)
    if sz % GRP:
        out.append(sz % GRP)
    return out

TILE_GROUPS = [_groups(sz) for sz in TILE_SIZES]
assert sum(sum(g) for g in TILE_GROUPS) == NSC

_CACHE = {}
LAST_RESULTS = None


def _patch_act_tables():
    """Pin Exp and Ln to the one table set containing both
    (natural_log_exp_and_others) so the ACT table loads exactly once.
    Table-set ids are positional, so keys/order are preserved."""
    import functools
    import concourse.hw_specs as hw_specs
    import concourse.bacc as bacc_mod
    import concourse.mybir as mybir

    if _CACHE.get("tables_patched"):
        return
    orig = hw_specs.get_activation_tables
    Exp = mybir.ActivationFunctionType.Exp
    Ln = mybir.ActivationFunctionType.Ln

    def patched(arch):
        out = {}
        for name, fns in orig(arch).items():
            if name != "natural_log_exp_and_others":
                fns = fns - {Exp, Ln}
            out[name] = fns
        return out

    cached = functools.cache(patched)
    hw_specs.get_activation_tables = cached
    bacc_mod.get_activation_tables = cached
    _CACHE["tables_patched"] = True


def _build_nc():
    import concourse.bacc as bacc
    import concourse.mybir as mybir
    import concourse.tile as tile

    _patch_act_tables()

    F32 = mybir.dt.float32
    BF16 = mybir.dt.bfloat16
    Exp = mybir.ActivationFunctionType.Exp
    Ln = mybir.ActivationFunctionType.Ln

    nc = bacc.Bacc("TRN2", target_bir_lowering=False, debug=False)
    xt_d = nc.dram_tensor("xt", [NTILES, 128, XT_TILE], BF16, kind="ExternalInput")
    # col 258 carries b1s as bf16 (a separate [128,1] f32 tensor lowers to
    # 128 tiny DMA descriptors = ~3us of ring time; b1 is ~0.01-scale so
    # bf16 bias costs ~1e-4 absolute - negligible)
    wpk_d = nc.dram_tensor("wpk", [128, 260], BF16, kind="ExternalInput")
    mag_d = nc.dram_tensor(
        "mag", [(NBANK + 3) // 4, 2, 4, 4, 512], BF16, kind="ExternalOutput")
    scr_d = nc.dram_tensor("scr", [128, 1], F32, kind="ExternalOutput")

    with tile.TileContext(nc) as tc:
        with (
            tc.tile_pool(name="wp", bufs=1) as wp,
            tc.tile_pool(name="xp", bufs=5) as xp,
            tc.tile_pool(name="zsp", bufs=3) as zsp,
            tc.tile_pool(name="hp", bufs=3) as hp,
            tc.tile_pool(name="mp", bufs=2) as mp,
            tc.tile_pool(name="zp", bufs=3, space="PSUM") as zp,
            tc.tile_pool(name="magp", bufs=2, space="PSUM") as magp,
        ):
            # weights on the gpsimd queue: wpk's [128,260] layout lowers
            # to 128 small descriptors whose issue+transfer measured ~14us
            # cold when queued ahead of the xt stream; the sync ring stays a
            # pure xt tile stream
            wpack = wp.tile([128, 260], BF16, tag="wpk")
            nc.gpsimd.dma_start(wpack[:], wpk_d[:])
            w1a = wpack[:, 0:128]
            w1b = wpack[:, 128:256]
            w2d = wpack[:, 256:258]
            b1s = wpack[:, 258:260]

            # dummy activation: triggers the one ACT-table load while the
            # first xt tile is still streaming in
            scr = wp.tile([128, 1], F32, tag="scr")
            nc.vector.memset(scr[:], 0.0)
            nc.scalar.activation(scr[:], scr[:], Exp)
            nc.gpsimd.dma_start(scr_d[:], scr[:])

            # pending mm2 work, delayed TWO tiles: [(h_tile, sc0, nsc), ...]
            pending = []
            mag_t = None
            stage = None

            def emit_mm2(pend):
                nonlocal mag_t, stage
                h_t, sc0, nsc = pend
                for si in range(nsc):
                    sc_a = sc0 + si
                    slot = sc_a % 4
                    if slot == 0:
                        mag_t = magp.tile([128, 512], F32, tag="magt")
                    off = 32 * slot
                    nc.tensor.matmul(
                        mag_t[off:off + 2, :], w2d,
                        h_t[:, si * 512:(si + 1) * 512],
                        start=True, stop=True, tile_position=(0, off),
                    )
                    if slot == 3 or sc_a == NSC - 1:
                        bank = sc_a // 4
                        if bank % 4 == 0:
                            stage = mp.tile([128, 2048], BF16, tag="magsb")
                            if bank + 3 >= NBANK:
                                nc.vector.memset(stage[:], 0.0)
                        cb = (bank % 4) * 512
                        nc.vector.tensor_copy(stage[:, cb:cb + 512], mag_t[:])
                        if bank % 4 == 3 or bank == NBANK - 1:
                            batch = bank // 4
                            nc.gpsimd.dma_start(mag_d[batch, 0], stage[0:128:32])
                            nc.gpsimd.dma_start(mag_d[batch, 1], stage[1:128:32])

            sc_abs = 0
            for ti, size in enumerate(TILE_SIZES):
                width = size * SC
                zw = size * 512
                xt = xp.tile([128, XT_TILE], BF16, tag="xt")
                nc.sync.dma_start(xt[:, :width], xt_d[ti, :, :width])
                zs = zsp.tile([128, XT_TILE // 2], F32, tag="zs")
                base = 0
                zoff = 0
                for gsz in TILE_GROUPS[ti]:
                    gw = gsz * 512
                    z = zp.tile([128, GRP * 512], F32, tag="z")
                    # mm1: A-halves with w1a stationary, then B-halves w1b
                    for s in range(gsz):
                        nc.tensor.matmul(
                            z[:, s * 512:(s + 1) * 512], w1a,
                            xt[:, base + s * SC:base + s * SC + 512],
                            start=True, stop=False,
                        )
                    for s in range(gsz):
                        nc.tensor.matmul(
                            z[:, s * 512:(s + 1) * 512], w1b,
                            xt[:, base + s * SC + 512:base + s * SC + 1024],
                            start=False, stop=True,
                        )
                    # DVE evacuates z to the wide SBUF staging buffer
                    nc.vector.tensor_copy(zs[:, zoff:zoff + gw], z[:, :gw])
                    zoff += gw
                    base += gsz * SC
                # pipelined mm2 of tile t-2: PE's wait on LN(t-2) happens
                # after it has produced everything ACT needs next
                if len(pending) == 2:
                    emit_mm2(pending.pop(0))
                # softplus for the whole tile: TWO wide ACT ops
                nc.scalar.activation(zs[:, :zw], zs[:, :zw], Exp, bias=b1s[:, 0:1])
                h_t = hp.tile([128, XT_TILE // 2], BF16, tag="h")
                nc.scalar.activation(h_t[:, :zw], zs[:, :zw], Ln, bias=1.0)
                pending.append((h_t, sc_abs, size))
                sc_abs += size
            for p in pending:
                emit_mm2(p)
    nc.compile()
    return nc


def _get_nc():
    if "nc" not in _CACHE:
        _CACHE["nc"] = _build_nc()
    return _CACHE["nc"]


def kernel(features, edge_vectors, edge_lengths, edge_index, w1, b1, w2, b2):
    global LAST_RESULTS
    from concourse.bass_utils import run_bass_kernel_spmd

    BF = ml_dtypes.bfloat16
    features = np.asarray(features, dtype=np.float32)
    edge_vectors = np.asarray(edge_vectors, dtype=np.float32)
    edge_lengths = np.asarray(edge_lengths, dtype=np.float32)
    edge_index = np.asarray(edge_index)
    w1 = np.asarray(w1, dtype=np.float32)
    b1 = np.asarray(b1, dtype=np.float32).reshape(-1)
    w2 = np.asarray(w2, dtype=np.float32).reshape(-1, 1)
    b2 = np.asarray(b2, dtype=np.float32).reshape(-1)

    # replicated small weights, padded for the stacked-z / block-diag tricks
    w1a = np.zeros((128, 128), np.float32)
    w1a[:, :H_DIM] = w1
    w1b = np.zeros((128, 128), np.float32)
    w1b[:, H_DIM:] = w1
    b1s = np.concatenate([b1, b1]).astype(np.float32).reshape(128, 1)
    w2d = np.zeros((128, 2), np.float32)
    w2d[:H_DIM, 0] = w2[:, 0]
    w2d[H_DIM:, 1] = w2[:, 0]
    b1c = np.concatenate([b1s, np.zeros((128, 1), np.float32)], axis=1)
    wpk = np.concatenate([w1a, w1b, w2d, b1c], axis=1).astype(BF)

    feats_bf = features.astype(BF)

    # shard edges contiguously across cores; per-core transposed bf16 panel
    in_maps = []
    for c in range(N_CORES):
        sl = slice(c * EC, (c + 1) * EC)
        panel = np.zeros((128, ECP), BF)
        panel[:, :EC] = feats_bf[sl].T
        xt = np.zeros((NTILES, 128, XT_TILE), BF)
        a = 0
        for ti, size in enumerate(TILE_SIZES):
            w = size * SC
            xt[ti, :, :w] = panel[:, a:a + w]
            a += w
        in_maps.append({"xt": xt, "wpk": wpk})

    nc = _get_nc()
    try:
        res = run_bass_kernel_spmd(nc, in_maps, core_ids=list(range(N_CORES)))
    except Exception:
        # one retry for transient runtime failures
        import time
        time.sleep(2.0)
        res = run_bass_kernel_spmd(nc, in_maps, core_ids=list(range(N_CORES)))
    LAST_RESULTS = res

    # decode mag: [batch, r, q, j, 512] bf16; sc = 16*batch + 4*j + q
    mag = np.empty(E_TOT, np.float32)
    for c in range(N_CORES):
        arr = np.asarray(res.results[c]["mag"], dtype=BF).astype(np.float32)
        mr = arr.transpose(0, 3, 2, 1, 4).reshape(-1, 2, 512)
        mag[c * EC:(c + 1) * EC] = mr[:NSC].reshape(-1)[:EC]

    # fold b2 and the shifted-softplus constant: h_ref = h_dev - log(2)
    mag = mag + (b2[0] - np.float32(np.log(2.0)) * w2.sum())

    center = edge_index[0].astype(np.int64)
    neigh = edge_index[1].astype(np.int64)

    # scatter-mean debias per center atom
    cnt = np.bincount(center, minlength=N_ATOMS).astype(np.float32)
    ssum = np.bincount(center, weights=mag.astype(np.float64), minlength=N_ATOMS)
    bias = (ssum / np.maximum(cnt, 1.0)).astype(np.float32)
    mag = mag - bias[center]

    # pair-averaged antisymmetric force assembly (see module docstring)
    unit = edge_vectors / edge_lengths[:, None]
    val = (0.5 * mag)[:, None] * unit  # [E, 3]
    forces = np.zeros((N_ATOMS, 3), np.float32)
    for k in range(3):
        fc = np.bincount(center, weights=val[:, k].astype(np.float64), minlength=N_ATOMS)
        fn = np.bincount(neigh, weights=val[:, k].astype(np.float64), minlength=N_ATOMS)
        forces[:, k] = (fc - fn).astype(np.float32)
    return forces
